# revision 22
# baseline (speedup 1.0000x reference)
"""Trainium2 Bass kernel for nn_BehaviorRegression (segment mean-pool + linear head).

Per batch row b (one NeuronCore each, 8 rows / 8 cores):
    pad_mask[t] = t >= lengths[b]
    tmark[t]    = TM if pad else time[b,t]
    S[m, :]     = sum_{t: tmark[t]==m} X[t, :]          (segment sums, m < TM)
    pooled      = S / max(cnt, 1)
    out[m, :]   = pooled[m] @ W.T + b_out               -> (TM, D)
    new_pad_mask[m] = (no t has raw time == m)

Device strategy (memory-bound; X is 16 MiB/core, everything else is tiny):
  - Stream X in 32 tiles of (128, 1024), natural layout (contiguous DMA).
  - VectorE builds a one-hot A (128 tokens x 512 bins) per tile with one
    tensor_scalar(is_equal) against a host-supplied iota row (padded tokens
    have tmark=512 which never matches -> contribute zero).
  - TensorE computes S.T = X.T @ A with X slices as the stationary operand,
    accumulating (128 h x 512 bins) x 8 h-chunks across all 32 tiles in the
    8 PSUM banks.
  - ScalarE evacuates S.T to SBUF; TensorE projects out.T = W @ S.T (2 x 512);
    VectorE applies 1/max(cnt,1) (commutes past W) and the bias; DMA out.
Host does only metadata work: bincounts over `time` (8x4096 ints) for the
mean divisor + new_pad_mask, plus input reshapes and the final transpose.
"""

import os
import numpy as np
from contextlib import ExitStack

B, T, H, TM, D = 8, 4096, 1024, 512, 2
P = 128
NT = T // P   # 32 token tiles
NH = H // P   # 8 h-chunks

_CACHE = {}


def _build_nc(repeat=1, mode="full"):
    # mode: "full" = real kernel; "dma" = X DMA only; "pe" = matmuls only
    # (single resident tile, no per-tile DMA). The micro modes exist to
    # partition measured HW time between the DMA and PE pipelines.
    import concourse.bacc as bacc
    import concourse.tile as tile
    from concourse import mybir

    f32 = mybir.dt.float32
    f32r = mybir.dt.float32r

    nc = bacc.Bacc("TRN2", target_bir_lowering=False, debug=False,
                   enable_asserts=False, num_devices=B)

    x = nc.dram_tensor("x", (T, H), f32r, kind="ExternalInput")
    tm = nc.dram_tensor("tm", (P, NT), f32, kind="ExternalInput")
    iota = nc.dram_tensor("iota", (P, TM), f32, kind="ExternalInput")
    wt = nc.dram_tensor("wt", (P, NH * D), f32, kind="ExternalInput")
    recip = nc.dram_tensor("recip", (D, TM), f32, kind="ExternalInput")
    bias = nc.dram_tensor("bias", (D, 1), f32, kind="ExternalInput")
    out = nc.dram_tensor("out", (D, TM), f32, kind="ExternalOutput")

    with tile.TileContext(nc) as tc, ExitStack() as ctx:
        consts = ctx.enter_context(tc.tile_pool(name="consts", bufs=1))
        xp = ctx.enter_context(tc.tile_pool(name="xp", bufs=6))
        ahp = ctx.enter_context(tc.tile_pool(name="ahp", bufs=6))
        ev = ctx.enter_context(tc.tile_pool(name="ev", bufs=2))
        ps = ctx.enter_context(tc.tile_pool(name="ps", bufs=8, space="PSUM"))

        iota_sb = consts.tile([P, TM], f32)
        nc.sync.dma_start(out=iota_sb, in_=iota[:, :])
        tm_sb = consts.tile([P, NT], f32)
        nc.sync.dma_start(out=tm_sb, in_=tm[:, :])
        wt_sb = consts.tile([P, NH * D], f32)
        nc.sync.dma_start(out=wt_sb, in_=wt[:, :])
        recip_sb = consts.tile([D, TM], f32)
        nc.sync.dma_start(out=recip_sb, in_=recip[:, :])
        bias_sb = consts.tile([D, 1], f32)
        nc.sync.dma_start(out=bias_sb, in_=bias[:, :])

        if mode == "dma":
            a_dummy = consts.tile([P, TM], f32r)
            nc.vector.tensor_scalar(a_dummy, iota_sb, 0.0, None,
                                    mybir.AluOpType.mult)
        if mode == "pe":
            x_res = consts.tile([P, H], f32r)
            nc.sync.dma_start(out=x_res, in_=x[0:P, :])
            a_res = consts.tile([P, TM], f32r)
            nc.vector.tensor_scalar(a_res, iota_sb, tm_sb[:, 0:1], None,
                                    mybir.AluOpType.is_equal)

        for _rep in range(repeat):
            # S.T accumulators: 8 x (128 h, 512 bins) = all 8 PSUM banks.
            st_ps = [ps.tile([P, TM], f32, tag="st", name=f"st_ps{j}")
                     for j in range(1 if mode == "dma" else NH)]

            for i in range(NT):
                if mode != "pe" and i % 2 == 0:
                    # One 1 MiB DMA covers token-tiles i and i+1: partition p
                    # holds row 128*i+p in cols [0,H) and row 128*(i+1)+p in
                    # cols [H,2H).
                    x_pair = xp.tile([P, 2, H], f32r, tag="x_pair", name=f"x_pair{i}")
                    src = x[i * P:(i + 2) * P, :].rearrange(
                        "(two p) h -> p two h", two=2)
                    eng = nc.sync if (i // 2) % 2 == 0 else nc.scalar
                    eng.dma_start(out=x_pair, in_=src)
                if mode != "pe":
                    x_tile = x_pair[:, i % 2, :]
                if mode == "dma":
                    # Cheap consumer so the DMA isn't dead code: N=1 matmul.
                    nc.tensor.matmul(st_ps[0], x_tile[:, 0:P],
                                     a_dummy, start=True, stop=True,
                                     skip_group_check=True)
                    continue
                if mode == "pe":
                    x_tile, a_tile = x_res, a_res
                else:
                    a_tile = ahp.tile([P, TM], f32r)
                    nc.vector.tensor_scalar(a_tile, iota_sb, tm_sb[:, i:i + 1],
                                            None, mybir.AluOpType.is_equal)
                for j in range(NH):
                    # float32r: fp32-storage matmul at 1 cycle/row (vs 4 for
                    # plain fp32). The moving operand is an exact 0/1 one-hot.
                    nc.tensor.matmul(st_ps[j], x_tile[:, j * P:(j + 1) * P],
                                     a_tile, start=(i == 0), stop=(i == NT - 1))

            st_sb = []
            for j in range(NH):
                s = ev.tile([P, TM], f32, tag=f"stsb{j}", name=f"st_sb{j}")
                src_ps = st_ps[0 if mode == "dma" else j]
                # Alternate evac engines so the kernel tail halves.
                if j % 2 == 0:
                    nc.scalar.copy(out=s, in_=src_ps)
                else:
                    nc.vector.tensor_copy(out=s, in_=src_ps)
                st_sb.append(s)

            out_ps = ps.tile([D, TM], f32, tag="st")
            for j in range(NH):
                nc.tensor.matmul(out_ps, wt_sb[:, j * D:(j + 1) * D], st_sb[j],
                                 start=(j == 0), stop=(j == NH - 1))

            res = ev.tile([D, TM], f32, tag="res")
            nc.vector.tensor_mul(res, out_ps, recip_sb)
            res2 = ev.tile([D, TM], f32, tag="res2")
            nc.vector.tensor_scalar(res2, res, bias_sb[:, 0:1], None,
                                    mybir.AluOpType.add)
            nc.sync.dma_start(out=out[:, :], in_=res2)

    nc.compile()
    return nc


def _get_nc(repeat=1, mode="full"):
    key = f"nc{repeat}_{mode}"
    if key not in _CACHE:
        _CACHE[key] = _build_nc(repeat, mode)
    return _CACHE[key]


def prep_in_maps(backbone_features, time, lengths, override_time, W, b_out):
    """Host metadata prep (tiny (8,4096) index tensors only) + input reshapes.

    Returns (in_maps, new_pad_mask)."""
    x = np.ascontiguousarray(np.asarray(backbone_features, dtype=np.float32))
    t = np.asarray(time).astype(np.int64)
    ln = np.asarray(lengths).astype(np.int64)
    tmv = int(override_time)
    assert x.shape == (B, T, H) and tmv == TM, (x.shape, tmv)
    W_ = np.asarray(W, dtype=np.float32)
    b_ = np.asarray(b_out, dtype=np.float32)

    pad = np.arange(T)[None, :] >= ln[:, None]
    tmark = np.where(pad, TM, t)
    cnt = np.stack([np.bincount(tmark[b], minlength=TM + 1)[:TM] for b in range(B)])
    recip = (1.0 / np.maximum(cnt, 1.0)).astype(np.float32)
    cnt2 = np.stack([np.bincount(t[b], minlength=TM) for b in range(B)])
    new_pad_mask = cnt2 == 0

    tm_in = np.ascontiguousarray(
        tmark.astype(np.float32).reshape(B, NT, P).transpose(0, 2, 1))
    iota_in = np.ascontiguousarray(
        np.broadcast_to(np.arange(TM, dtype=np.float32), (P, TM)))
    wt_in = np.ascontiguousarray(
        W_.T.reshape(NH, P, D).transpose(1, 0, 2).reshape(P, NH * D))
    recip_in = np.ascontiguousarray(
        np.broadcast_to(recip[:, None, :], (B, D, TM)))
    bias_in = np.ascontiguousarray(b_.reshape(D, 1))

    in_maps = [{
        "x": x[b],
        "tm": tm_in[b],
        "iota": iota_in,
        "wt": wt_in,
        "recip": recip_in[b],
        "bias": bias_in,
    } for b in range(B)]
    return in_maps, new_pad_mask


def kernel(backbone_features, time, lengths, override_time, W, b_out):
    from concourse.bass_utils import run_bass_kernel_spmd

    in_maps, new_pad_mask = prep_in_maps(
        backbone_features, time, lengths, override_time, W, b_out)
    nc = _get_nc()
    trace = bool(int(os.environ.get("KERNEL_TRACE", "0")))
    res = run_bass_kernel_spmd(nc, in_maps, core_ids=list(range(B)), trace=trace)
    _CACHE["last_results"] = res

    out_t = np.stack([res.results[b]["out"] for b in range(B)])   # (B, D, TM)
    out = np.ascontiguousarray(out_t.transpose(0, 2, 1))          # (B, TM, D)
    return out, new_pad_mask


# revision 24
# speedup vs baseline: 1.1644x; 1.1644x over previous
"""Trainium2 Bass kernel for nn_BehaviorRegression (segment mean-pool + linear head).

Per batch row b (one NeuronCore each, 8 rows / 8 cores):
    pad_mask[t] = t >= lengths[b]
    tmark[t]    = TM if pad else time[b,t]
    S[m, :]     = sum_{t: tmark[t]==m} X[t, :]          (segment sums, m < TM)
    pooled      = S / max(cnt, 1)
    out[m, :]   = pooled[m] @ W.T + b_out               -> (TM, D)
    new_pad_mask[m] = (no t has raw time == m)

Device strategy (memory-bound; X is 16 MiB/core, everything else is tiny):
  - Stream X in 32 tiles of (128, 1024), natural layout (contiguous DMA).
  - VectorE builds a one-hot A (128 tokens x 512 bins) per tile with one
    tensor_scalar(is_equal) against a host-supplied iota row (padded tokens
    have tmark=512 which never matches -> contribute zero).
  - TensorE computes S.T = X.T @ A with X slices as the stationary operand,
    accumulating (128 h x 512 bins) x 8 h-chunks across all 32 tiles in the
    8 PSUM banks.
  - ScalarE evacuates S.T to SBUF; TensorE projects out.T = W @ S.T (2 x 512);
    VectorE applies 1/max(cnt,1) (commutes past W) and the bias; DMA out.
Host does only metadata work: bincounts over `time` (8x4096 ints) for the
mean divisor + new_pad_mask, plus input reshapes and the final transpose.
"""

import os
import numpy as np
from contextlib import ExitStack

B, T, H, TM, D = 8, 4096, 1024, 512, 2
P = 128
NT = T // P   # 32 token tiles
NH = H // P   # 8 h-chunks

_CACHE = {}


def _build_nc(repeat=1, mode="full"):
    # mode: "full" = real kernel; "dma" = X DMA only; "pe" = matmuls only
    # (single resident tile, no per-tile DMA). The micro modes exist to
    # partition measured HW time between the DMA and PE pipelines.
    import concourse.bacc as bacc
    import concourse.tile as tile
    from concourse import mybir

    f32 = mybir.dt.float32
    f32r = mybir.dt.float32r

    nc = bacc.Bacc("TRN2", target_bir_lowering=False, debug=False,
                   enable_asserts=False, num_devices=B)

    import concourse.bass as bass_mod
    x = nc.dram_tensor("x", (T, H), f32r, kind="ExternalInput")
    if mode == "gather":
        idx = nc.dram_tensor("idx", (P, NT), mybir.dt.int32, kind="ExternalInput")
    tm = nc.dram_tensor("tm", (P, NT), f32, kind="ExternalInput")
    iota = nc.dram_tensor("iota", (P, TM), f32, kind="ExternalInput")
    wt = nc.dram_tensor("wt", (P, NH * D), f32, kind="ExternalInput")
    recip = nc.dram_tensor("recip", (D, TM), f32, kind="ExternalInput")
    bias = nc.dram_tensor("bias", (D, 1), f32, kind="ExternalInput")
    out = nc.dram_tensor("out", (D, TM), f32, kind="ExternalOutput")

    with tile.TileContext(nc) as tc, ExitStack() as ctx:
        consts = ctx.enter_context(tc.tile_pool(name="consts", bufs=1))
        xp = ctx.enter_context(tc.tile_pool(name="xp", bufs=6))
        ahp = ctx.enter_context(tc.tile_pool(name="ahp", bufs=6))
        ev = ctx.enter_context(tc.tile_pool(name="ev", bufs=2))
        ps = ctx.enter_context(tc.tile_pool(name="ps", bufs=8, space="PSUM"))

        iota_sb = consts.tile([P, TM], f32)
        nc.sync.dma_start(out=iota_sb, in_=iota[:, :])
        tm_sb = consts.tile([P, NT], f32)
        nc.sync.dma_start(out=tm_sb, in_=tm[:, :])
        wt_sb = consts.tile([P, NH * D], f32)
        nc.sync.dma_start(out=wt_sb, in_=wt[:, :])
        recip_sb = consts.tile([D, TM], f32)
        nc.sync.dma_start(out=recip_sb, in_=recip[:, :])
        bias_sb = consts.tile([D, 1], f32)
        nc.sync.dma_start(out=bias_sb, in_=bias[:, :])

        if mode == "gather":
            idx_sb = consts.tile([P, NT], mybir.dt.int32)
            nc.sync.dma_start(out=idx_sb, in_=idx[:, :])
        if mode in ("dma", "gather"):
            a_dummy = consts.tile([P, TM], f32r)
            nc.vector.tensor_scalar(a_dummy, iota_sb, 0.0, None,
                                    mybir.AluOpType.mult)
        if mode == "pe":
            x_res = consts.tile([P, H], f32r)
            nc.sync.dma_start(out=x_res, in_=x[0:P, :])
            a_res = consts.tile([P, TM], f32r)
            nc.vector.tensor_scalar(a_res, iota_sb, tm_sb[:, 0:1], None,
                                    mybir.AluOpType.is_equal)

        for _rep in range(repeat):
            # S.T accumulators: 8 x (128 h, 512 bins) = all 8 PSUM banks.
            st_ps = [ps.tile([P, TM], f32, tag="st", name=f"st_ps{j}")
                     for j in range(1 if mode in ("dma", "gather") else NH)]

            for i in range(NT):
                if mode == "gather":
                    x_tile = xp.tile([P, H], f32r, tag="x_pair", name=f"xg{i}")
                    nc.gpsimd.indirect_dma_start(
                        out=x_tile[:, :], out_offset=None, in_=x[:, :],
                        in_offset=bass_mod.IndirectOffsetOnAxis(
                            ap=idx_sb[:, i:i + 1], axis=0))
                    nc.tensor.matmul(st_ps[0], x_tile[:, 0:P],
                                     a_dummy, start=True, stop=True,
                                     skip_group_check=True)
                    continue
                if mode != "pe" and i % 2 == 0:
                    # One 1 MiB DMA covers token-tiles i and i+1: partition p
                    # holds row 128*i+p in cols [0,H) and row 128*(i+1)+p in
                    # cols [H,2H).
                    x_pair = xp.tile([P, 2, H], f32r, tag="x_pair", name=f"x_pair{i}")
                    src = x[i * P:(i + 2) * P, :].rearrange(
                        "(two p) h -> p two h", two=2)
                    eng = nc.sync if (i // 2) % 2 == 0 else nc.scalar
                    eng.dma_start(out=x_pair, in_=src)
                if mode != "pe":
                    x_tile = x_pair[:, i % 2, :]
                if mode == "dma":
                    # Cheap consumer so the DMA isn't dead code: N=1 matmul.
                    nc.tensor.matmul(st_ps[0], x_tile[:, 0:P],
                                     a_dummy, start=True, stop=True,
                                     skip_group_check=True)
                    continue
                if mode == "pe":
                    x_tile, a_tile = x_res, a_res
                else:
                    a_tile = ahp.tile([P, TM], f32r)
                    nc.vector.tensor_scalar(a_tile, iota_sb, tm_sb[:, i:i + 1],
                                            None, mybir.AluOpType.is_equal)
                for j in range(NH):
                    # float32r: fp32-storage matmul at 1 cycle/row (vs 4 for
                    # plain fp32). The moving operand is an exact 0/1 one-hot.
                    nc.tensor.matmul(st_ps[j], x_tile[:, j * P:(j + 1) * P],
                                     a_tile, start=(i == 0), stop=(i == NT - 1))

            st_sb = []
            for j in range(NH):
                s = ev.tile([P, TM], f32, tag=f"stsb{j}", name=f"st_sb{j}")
                src_ps = st_ps[0 if mode in ("dma", "gather") else j]
                # Alternate evac engines so the kernel tail halves.
                if j % 2 == 0:
                    nc.scalar.copy(out=s, in_=src_ps)
                else:
                    nc.vector.tensor_copy(out=s, in_=src_ps)
                st_sb.append(s)

            out_ps = ps.tile([D, TM], f32, tag="st")
            for j in range(NH):
                nc.tensor.matmul(out_ps, wt_sb[:, j * D:(j + 1) * D], st_sb[j],
                                 start=(j == 0), stop=(j == NH - 1))

            res = ev.tile([D, TM], f32, tag="res")
            nc.vector.tensor_mul(res, out_ps, recip_sb)
            res2 = ev.tile([D, TM], f32, tag="res2")
            nc.vector.tensor_scalar(res2, res, bias_sb[:, 0:1], None,
                                    mybir.AluOpType.add)
            nc.sync.dma_start(out=out[:, :], in_=res2)

    nc.compile()
    return nc


def _get_nc(repeat=1, mode="full"):
    key = f"nc{repeat}_{mode}"
    if key not in _CACHE:
        _CACHE[key] = _build_nc(repeat, mode)
    return _CACHE[key]


def prep_in_maps(backbone_features, time, lengths, override_time, W, b_out):
    """Host metadata prep (tiny (8,4096) index tensors only) + input reshapes.

    Returns (in_maps, new_pad_mask)."""
    x = np.ascontiguousarray(np.asarray(backbone_features, dtype=np.float32))
    t = np.asarray(time).astype(np.int64)
    ln = np.asarray(lengths).astype(np.int64)
    tmv = int(override_time)
    assert x.shape == (B, T, H) and tmv == TM, (x.shape, tmv)
    W_ = np.asarray(W, dtype=np.float32)
    b_ = np.asarray(b_out, dtype=np.float32)

    pad = np.arange(T)[None, :] >= ln[:, None]
    tmark = np.where(pad, TM, t)
    cnt = np.stack([np.bincount(tmark[b], minlength=TM + 1)[:TM] for b in range(B)])
    recip = (1.0 / np.maximum(cnt, 1.0)).astype(np.float32)
    cnt2 = np.stack([np.bincount(t[b], minlength=TM) for b in range(B)])
    new_pad_mask = cnt2 == 0

    tm_in = np.ascontiguousarray(
        tmark.astype(np.float32).reshape(B, NT, P).transpose(0, 2, 1))
    iota_in = np.ascontiguousarray(
        np.broadcast_to(np.arange(TM, dtype=np.float32), (P, TM)))
    wt_in = np.ascontiguousarray(
        W_.T.reshape(NH, P, D).transpose(1, 0, 2).reshape(P, NH * D))
    recip_in = np.ascontiguousarray(
        np.broadcast_to(recip[:, None, :], (B, D, TM)))
    bias_in = np.ascontiguousarray(b_.reshape(D, 1))

    in_maps = [{
        "x": x[b],
        "tm": tm_in[b],
        "iota": iota_in,
        "wt": wt_in,
        "recip": recip_in[b],
        "bias": bias_in,
    } for b in range(B)]
    return in_maps, new_pad_mask


def kernel(backbone_features, time, lengths, override_time, W, b_out):
    from concourse.bass_utils import run_bass_kernel_spmd

    in_maps, new_pad_mask = prep_in_maps(
        backbone_features, time, lengths, override_time, W, b_out)
    nc = _get_nc()
    trace = bool(int(os.environ.get("KERNEL_TRACE", "0")))
    res = run_bass_kernel_spmd(nc, in_maps, core_ids=list(range(B)), trace=trace)
    _CACHE["last_results"] = res

    out_t = np.stack([res.results[b]["out"] for b in range(B)])   # (B, D, TM)
    out = np.ascontiguousarray(out_t.transpose(0, 2, 1))          # (B, TM, D)
    return out, new_pad_mask
